# revision 1
# baseline (speedup 1.0000x reference)
"""MLA-style causal self-attention on 8 Trainium2 NeuronCores.

Sharding: tensor-parallel over heads (2 heads/core). W_qdec column-split,
W_out row-split by head; x / W_qkv / rope tables replicated. Each core
returns a partial out^T [E, T]; the host sums the 8 partials (the standard
gather for row-split tensor parallelism) and transposes.

Device dataflow (everything kept "transposed" so matmuls contract over the
partition dim with no activation transposes except c_kv -> v):
  A:    latents^T = W_qkv^T @ x^T, 13 output-column groups per 256-wide
        T chunk (kv+rope -> SBUF residents, c_q -> DRAM scratch).
  preC: rope k_r^T (rotate-half via a 64x64 permutation matmul, so no
        DMA sits on the critical path); v = (c_kv^T)^T via PE transposes.
  B/C fused per head h and 512-query chunk i4:
        q^T chunk = W_qdec_h^T @ c_q^T chunk (scale folded in), rope
        group first so the rope chain overlaps the nope groups, then
        flash-style causal attention: st[k,q] accumulated over 5 key
        matmuls, exp on ScalarE, edge-masked, denominator via
        ones-matmul over a DVE-accumulated sum, y^T[d,q] accumulated in
        4 PSUM banks, normalized, staged to per-chunk DRAM tiles.
  D:    out^T = W_out_c^T @ y^T -> HBM (kc-major so the stationary
        operand is reused across 4 consecutive matmuls).
All big matmuls run in float32r (full PE rate, fp32 PSUM accumulate).
"""

import math
from contextlib import ExitStack

import numpy as np

import concourse.bass as bass
import concourse.tile as tile
from concourse import bacc, mybir
from concourse.bass_utils import run_bass_kernel_spmd
from concourse.masks import make_identity

F32 = mybir.dt.float32
F32R = mybir.dt.float32r
AF = mybir.ActivationFunctionType

# Problem constants (hardcoded per harness contract)
T_FULL = 2048
E = 2048          # n_embd
KV = 512          # kv low rank == head size
QL = 1024         # q low rank
RH = 64           # rope head size
QKH = KV + RH     # 576
NH = 16
NCORES = 8
HPC = NH // NCORES  # heads per core
SCALE = 1.0 / math.sqrt(float(KV))

P = 128


def _make_rot64(nc, pool):
    """RT [64, 64] permutation with RT[x, y] = 1 iff x == (y+32) % 64, so
    matmul(out, lhsT=RT, rhs=src) gives out[d] = src[(d+32) % 64]."""
    rt0 = pool.tile([RH, RH], F32, tag="rt0")
    nc.gpsimd.memset(rt0[:], 0.0)
    # fill 1 where x - y - 32 == 0
    nc.gpsimd.affine_select(
        out=rt0[:], in_=rt0[:], compare_op=mybir.AluOpType.not_equal,
        fill=1.0, base=-32, channel_multiplier=1, pattern=[[-1, RH]],
    )
    # fill 1 where x - y + 32 == 0
    nc.gpsimd.affine_select(
        out=rt0[:], in_=rt0[:], compare_op=mybir.AluOpType.not_equal,
        fill=1.0, base=32, channel_multiplier=1, pattern=[[-1, RH]],
    )
    rt = pool.tile([RH, RH], F32R, tag="rt")
    nc.vector.tensor_copy(rt[:], rt0[:])
    return rt


def build_kernel(T=T_FULL):
    """Build the single-core program (SPMD across 8 cores via per-core data)."""
    assert T % 512 == 0
    NT512 = T // 512
    NT256 = T // 256
    NKT = T // P          # key tiles of 128
    EK = E // P           # 16 contraction chunks for phase A

    nc = bacc.Bacc("TRN2", target_bir_lowering=False, debug=False,
                   num_devices=NCORES)

    xT = nc.dram_tensor("xT", [E, T], F32R, kind="ExternalInput").ap()
    wqkv = nc.dram_tensor("wqkv", [E, QKH + QL], F32R, kind="ExternalInput").ap()
    wqdec = nc.dram_tensor("wqdec", [QL, HPC * QKH], F32R, kind="ExternalInput").ap()
    wout = nc.dram_tensor("wout", [HPC * KV, E], F32R, kind="ExternalInput").ap()
    cosd = nc.dram_tensor("cosT", [RH, T], F32, kind="ExternalInput").ap()
    sind = nc.dram_tensor("sinT", [RH, T], F32, kind="ExternalInput").ap()
    outT = nc.dram_tensor("outT", [E, T], F32, kind="ExternalOutput").ap()

    xT_r = xT.rearrange("(ko p) t -> p ko t", p=P)
    wq_r = wqkv.rearrange("(ko p) m -> p ko m", p=P)

    # latent column groups: 4x128 c_kv, 1x64 k_r, 8x128 c_q
    groups = [(i * P, P) for i in range(KV // P)] + [(KV, RH)] + [
        (QKH + i * P, P) for i in range(QL // P)
    ]

    with tile.TileContext(nc) as tc, ExitStack() as ctx:
        dram = ctx.enter_context(tc.tile_pool(name="dram", bufs=1, space="DRAM"))
        cst = ctx.enter_context(tc.tile_pool(name="cst", bufs=1))
        kvp = ctx.enter_context(tc.tile_pool(name="kvp", bufs=1))
        pp = ctx.enter_context(tc.tile_pool(name="pp", bufs=2, space="PSUM"))
        ppy = ctx.enter_context(tc.tile_pool(name="ppy", bufs=1, space="PSUM"))

        cqd = dram.tile([QL, T], F32R, tag="cqd")        # c_q^T scratch
        # y^T scratch, one DRAM tile per (head, d-chunk) so phase D can
        # stream each back as soon as that head's chunk is complete
        yTd = [dram.tile([P, T], F32R, tag=f"yTd{k}", name=f"yTd{k}")
               for k in range(HPC * KV // P)]
        cq_r = cqd[:].rearrange("(ko p) t -> p ko t", p=P)

        # ---- global constants / kv residents ----
        ident0 = cst.tile([P, P], F32, tag="ident0")
        make_identity(nc, ident0[:])
        ident = cst.tile([P, P], F32R, tag="ident")
        nc.vector.tensor_copy(ident[:], ident0[:])
        ones0 = cst.tile([P, 1], F32, tag="ones0")
        nc.gpsimd.memset(ones0[:], 1.0)
        ones_col = cst.tile([P, 1], F32R, tag="ones")
        nc.vector.tensor_copy(ones_col[:], ones0[:])

        ckvT = kvp.tile([P, KV // P, T], F32R, tag="ckvT")  # c_kv^T
        krT = kvp.tile([RH, T], F32R, tag="krT")            # k_r^T (pre-rope)
        krT2 = kvp.tile([RH, T], F32R, tag="krT2")          # k_r^T (roped)

        # ================= Phase A: latents^T = W_qkv^T @ x^T ==============
        with ExitStack() as actx:
            aw = actx.enter_context(tc.tile_pool(name="aw", bufs=1))
            asr = actx.enter_context(tc.tile_pool(name="asr", bufs=2))
            astp = actx.enter_context(tc.tile_pool(name="astp", bufs=3))

            EH = EK // 2
            wqt = []
            for gi, (c0, M) in enumerate(groups):
                wa = aw.tile([P, EH, M], F32R, tag=f"wqa{gi}", name=f"wqa{gi}")
                wb = aw.tile([P, EH, M], F32R, tag=f"wqb{gi}", name=f"wqb{gi}")
                e0, e1 = ((nc.gpsimd, nc.scalar) if gi % 2 == 0
                          else (nc.scalar, nc.gpsimd))
                if gi % 3 == 1:
                    # borrow the lightly-loaded sync queue for a third of
                    # the second halves: the first-chunk group consumption
                    # outruns two queues during the cold ramp
                    e1 = nc.sync
                e0.dma_start(wa[:], wq_r[:, 0:EH, c0 : c0 + M])
                e1.dma_start(wb[:], wq_r[:, EH:EK, c0 : c0 + M])
                wqt.append((wa, wb))

            for tcc in range(NT256):
                tsl = slice(tcc * 256, (tcc + 1) * 256)
                xta = asr.tile([P, EH, 256], F32R, tag="xta", name="xta")
                nc.sync.dma_start(xta[:], xT_r[:, 0:EH, tsl])
                xtb = asr.tile([P, EH, 256], F32R, tag="xtb", name="xtb")
                nc.sync.dma_start(xtb[:], xT_r[:, EH:EK, tsl])
                for gi, (c0, M) in enumerate(groups):
                    ps = pp.tile([P, 256], F32, tag="mm", name="psA")
                    for kc in range(EK):
                        xs = xta[:, kc, :] if kc < EH else xtb[:, kc - EH, :]
                        ws = (wqt[gi][0][:, kc, :] if kc < EH
                              else wqt[gi][1][:, kc - EH, :])
                        nc.tensor.matmul(
                            ps[:M], ws, xs,
                            start=(kc == 0), stop=(kc == EK - 1),
                        )
                    if c0 < KV:
                        nc.vector.tensor_copy(ckvT[:, c0 // P, tsl], ps[:])
                    elif c0 == KV:
                        nc.vector.tensor_copy(krT[:, tsl], ps[:RH])
                    else:
                        st = astp.tile([P, 256], F32R, tag="ast", name="ast")
                        nc.vector.tensor_copy(st[:], ps[:])
                        nc.scalar.dma_start(
                            cqd[c0 - QKH : c0 - QKH + M, tsl], st[:]
                        )

        # ============ Phases preC + fused B/C =============================
        with ExitStack() as bctx:
            bcp = bctx.enter_context(tc.tile_pool(name="bcp", bufs=1))
            bcs = bctx.enter_context(tc.tile_pool(name="bcs", bufs=2))

            # BC-scope constants: rope tables, rotation matrix, edge masks
            rt = _make_rot64(nc, bcp)
            masks = []
            for r in range(4):
                m0 = bcp.tile([P, 512], F32, tag=f"mask{r}", name=f"mask{r}")
                nc.gpsimd.memset(m0[:], 1.0)
                nc.gpsimd.affine_select(
                    out=m0[:], in_=m0[:], compare_op=mybir.AluOpType.is_ge,
                    fill=0.0, base=-P * r, channel_multiplier=-1,
                    pattern=[[1, 512]],
                )
                masks.append(m0)
            cosT = bcp.tile([RH, T], F32, tag="cosT")
            nc.sync.dma_start(cosT[:], cosd[:])
            ssinT = bcp.tile([RH, T], F32, tag="ssinT")
            nc.sync.dma_start(ssinT[:], sind[:])
            nc.vector.tensor_scalar_mul(
                ssinT[0 : RH // 2, :], ssinT[0 : RH // 2, :], -1.0
            )

            # rope k_r via permutation matmul, chunked
            for tcc in range(NT512):
                tsl = slice(tcc * 512, (tcc + 1) * 512)
                pr = pp.tile([RH, 512], F32, tag="mm", name="prk")
                nc.tensor.matmul(pr[:], rt[:], krT[:, tsl],
                                 start=True, stop=True)
                nc.vector.tensor_mul(krT2[:, tsl], krT[:, tsl], cosT[:, tsl])
                rot = bcs.tile([RH, 512], F32, tag="rot", name="rotk")
                nc.vector.tensor_mul(rot[:], pr[:], ssinT[:, tsl])
                nc.vector.tensor_add(krT2[:, tsl], krT2[:, tsl], rot[:])

            # v[t, d] via PE transposes of c_kv^T
            v = bcp.tile([P, NKT, KV], F32R, tag="v")
            with tc.tile_pool(name="ptr", bufs=2, space="PSUM") as ptr:
                for dc in range(KV // P):
                    for tt in range(NKT):
                        pt = ptr.tile([P, P], F32R, tag="tr", name="pt")
                        nc.tensor.transpose(
                            pt[:], ckvT[:, dc, tt * P : (tt + 1) * P], ident[:]
                        )
                        nc.vector.tensor_copy(
                            v[:, tt, dc * P : (dc + 1) * P], pt[:]
                        )
            pden = bctx.enter_context(
                tc.tile_pool(name="pden", bufs=2, space="PSUM")
            )

            # rope group first so its chain overlaps the nope matmul groups
            qgroups = [(KV, RH)] + [(i * P, P) for i in range(KV // P)]
            for h in range(HPC):
                wqd = bcp.tile([P, QL // P, QKH], F32R, tag="wqd", name="wqd")
                nc.gpsimd.dma_start(
                    wqd[:],
                    wqdec.rearrange("(ko p) m -> p ko m", p=P)[
                        :, :, h * QKH : (h + 1) * QKH
                    ],
                )
                for i4 in range(NT512):
                    qsl = slice(i4 * 512, (i4 + 1) * 512)
                    # ---- B chunk: q^T for queries i4 (SCALE folded in) ----
                    cq = bcs.tile([P, QL // P, 512], F32R, tag="cq", bufs=1,
                                  name="cq")
                    nc.sync.dma_start(cq[:], cq_r[:, :, qsl])
                    qTc = [bcs.tile([P, 512], F32R, tag=f"qTc{i}",
                                    name=f"qTc{i}") for i in range(KV // P)]
                    qrRaw = bcs.tile([RH, 512], F32R, tag="qrRaw", name="qrRaw")
                    qrT = bcs.tile([RH, 512], F32R, tag="qrT", name="qrT")
                    for (m0, M) in qgroups:
                        ps = pp.tile([P, 512], F32, tag="mm", name="psB")
                        for kc in range(QL // P):
                            nc.tensor.matmul(
                                ps[:M], wqd[:, kc, m0 : m0 + M], cq[:, kc, :],
                                start=(kc == 0), stop=(kc == QL // P - 1),
                            )
                        if m0 < KV:
                            nc.vector.tensor_scalar_mul(
                                qTc[m0 // P][:], ps[:], SCALE
                            )
                        else:
                            # rope group runs first; do its whole chain now so
                            # the DVE work hides under the four d-groups
                            nc.vector.tensor_scalar_mul(qrRaw[:], ps[:RH], SCALE)
                            pr = pp.tile([RH, 512], F32, tag="mm", name="prq")
                            nc.tensor.matmul(pr[:], rt[:], qrRaw[:],
                                             start=True, stop=True)
                            nc.vector.tensor_mul(qrT[:], qrRaw[:], cosT[:, qsl])
                            rot = bcs.tile([RH, 512], F32, tag="rot",
                                           name="rotq")
                            nc.vector.tensor_mul(rot[:], pr[:], ssinT[:, qsl])
                            nc.vector.tensor_add(qrT[:], qrT[:], rot[:])

                    # ---- C chunk: causal attention for queries i4 ----
                    nj = 4 * i4 + 4
                    psden = pden.tile([1, 512], F32, tag="den", name="psden")
                    psy = [ppy.tile([P, 512], F32, tag=f"y{dc}",
                                    name=f"psy{dc}")
                           for dc in range(KV // P)]
                    for j in range(nj):
                        ksl = slice(j * P, (j + 1) * P)
                        ps = pp.tile([P, 512], F32, tag="mm", name="psS")
                        for dc in range(KV // P):
                            nc.tensor.matmul(
                                ps[:], ckvT[:, dc, ksl], qTc[dc][:],
                                start=(dc == 0), stop=False,
                            )
                        nc.tensor.matmul(
                            ps[:], krT2[:, ksl], qrT[:],
                            start=False, stop=True,
                        )
                        se = bcs.tile([P, 512], F32R, tag="se", bufs=3,
                                      name="se")
                        nc.scalar.activation(se[:], ps[:], AF.Exp)
                        r = j - 4 * i4
                        if r >= 0:
                            nc.vector.tensor_mul(se[:], se[:], masks[r][:])
                        nc.tensor.matmul(psden[:], ones_col[:], se[:],
                                         start=(j == 0), stop=(j == nj - 1))
                        for dc in range(KV // P):
                            nc.tensor.matmul(
                                psy[dc][:], v[:, j, dc * P : (dc + 1) * P],
                                se[:],
                                start=(j == 0), stop=(j == nj - 1),
                            )
                    deninv = bcs.tile([1, 512], F32, tag="deninv",
                                      name="deninv")
                    nc.vector.reciprocal_approx_fast(out=deninv[:],
                                                     in_=psden[:])
                    denb = bcs.tile([P, 512], F32, tag="denb", name="denb")
                    nc.gpsimd.partition_broadcast(denb[:], deninv[:])
                    for dc in range(KV // P):
                        yst = bcs.tile([P, 512], F32R, tag="yst", name="yst")
                        nc.vector.tensor_mul(yst[:], psy[dc][:], denb[:])
                        nc.scalar.dma_start(
                            yTd[h * (KV // P) + dc][:, qsl], yst[:]
                        )

        # ================= Phase D: out^T = W_out_c^T @ y^T ===============
        with ExitStack() as dctx:
            dyp = dctx.enter_context(tc.tile_pool(name="dyp", bufs=1))
            dwp = dctx.enter_context(tc.tile_pool(name="dwp", bufs=2))
            dst = dctx.enter_context(tc.tile_pool(name="dst", bufs=3))

            DK = HPC * KV // P  # 8 contraction chunks
            yT_sb = []
            for k in range(0, DK):
                yk = dyp.tile([P, T], F32R, tag=f"yT{k}", name=f"yT{k}")
                nc.gpsimd.dma_start(yk[:], yTd[k][:])
                yT_sb.append(yk)
            wo_r = wout.rearrange("(ko p) e -> p ko e", p=P)
            for mc in range(E // P):
                wo = dwp.tile([P, DK, P], F32R, tag="wo", name="wo")
                nc.sync.dma_start(wo[:], wo_r[:, :, mc * P : (mc + 1) * P])
                psD = [ppy.tile([P, 512], F32, tag=f"y{tcc}", name=f"psD{tcc}")
                       for tcc in range(NT512)]
                # kc-major: the stationary operand is reused across the 4
                # consecutive matmuls, letting LDWEIGHTS pull-ahead hide
                for kc in range(DK):
                    for tcc in range(NT512):
                        nc.tensor.matmul(
                            psD[tcc][:], wo[:, kc, :],
                            yT_sb[kc][:, tcc * 512 : (tcc + 1) * 512],
                            start=(kc == 0), stop=(kc == DK - 1),
                        )
                    if kc == DK - 1:
                        # copies chase the closing matmuls, split DVE/ACT
                        for tcc in range(NT512):
                            ost = dst.tile([P, 512], F32, tag=f"ost{tcc % 2}",
                                           name="ost")
                            if tcc % 2 == 0:
                                nc.vector.tensor_copy(ost[:], psD[tcc][:])
                            else:
                                nc.scalar.copy(ost[:], psD[tcc][:])
                            nc.scalar.dma_start(
                                outT[mc * P : (mc + 1) * P,
                                     tcc * 512 : (tcc + 1) * 512], ost[:]
                            )

    nc.compile()
    return nc


_NC_CACHE = {}


def _get_nc(T=T_FULL):
    if T not in _NC_CACHE:
        _NC_CACHE[T] = build_kernel(T)
    return _NC_CACHE[T]


def make_in_maps(x, cos, sin, W_qkv, W_qdec, W_out):
    """Host-side sharding/layout: transpose activations & tables, slice the
    head-parallel weights. Returns one input dict per core."""
    xT = np.ascontiguousarray(np.asarray(x)[0].T)
    cosT = np.ascontiguousarray(np.asarray(cos).T)
    sinT = np.ascontiguousarray(np.asarray(sin).T)
    W_qkv = np.ascontiguousarray(np.asarray(W_qkv))
    W_qdec = np.asarray(W_qdec)
    W_out = np.asarray(W_out)
    in_maps = []
    for c in range(NCORES):
        in_maps.append({
            "xT": xT,
            "wqkv": W_qkv,
            "wqdec": np.ascontiguousarray(
                W_qdec[:, c * HPC * QKH : (c + 1) * HPC * QKH]
            ),
            "wout": np.ascontiguousarray(
                W_out[c * HPC * KV : (c + 1) * HPC * KV]
            ),
            "cosT": cosT,
            "sinT": sinT,
        })
    return in_maps


def kernel(x, cos, sin, W_qkv, W_qdec, W_out, _trace=False, _tmpdir=None):
    T = np.asarray(x).shape[1]
    nc = _get_nc(T)
    in_maps = make_in_maps(x, cos, sin, W_qkv, W_qdec, W_out)
    res = run_bass_kernel_spmd(
        nc, in_maps, core_ids=list(range(NCORES)),
        trace=_trace, tmpdir=_tmpdir,
    )
    out = np.zeros((E, T), np.float32)
    for r in res.results:
        out += r["outT"]
    kernel.last_results = res
    return np.ascontiguousarray(out.T)[None].astype(np.float32)



# revision 17
# speedup vs baseline: 1.1578x; 1.1578x over previous
"""MLA-style causal self-attention on 8 Trainium2 NeuronCores.

v5: tensor-parallel over heads (2/core) for attention + output proj, with the
latent projection (x @ W_qkv) SHARDED over the sequence across cores and
AllGathered (collective, ~80us, hidden under phase-B weight loads + local
work). All matmul operands in bf16 (1 cycle/row at any free size, half the
DMA/SBUF of f32r); PSUM accumulation stays fp32.

Per-core device dataflow (everything transposed: contraction on partitions):
  A-local: latT[1664, 256] = Wlat^T @ xT_slice for this core's T-slice
           (cols: ckv 512 | kr 64 + pad 64 | c_q 1024); k-rope applied
           locally via a 128x128 block-diag rotate-half permutation matmul;
           staged to DRAM and AllGathered -> agout [8*1664, 256].
  B: per 512-query chunk, q^T = W_qdec_c^T @ c_q^T from the gathered c_q
     (contraction 1024, SCALE folded into W_qdec host-side), rope group
     first ([qr_h0|qr_h1] packed as one 128-row group).
  C: per (chunk i4, head h): flash-style causal attention. Scores per
     128-key tile: 4 ckv matmuls + 1 rope matmul -> exp on ACT (bf16 out)
     -> stair-mask (diag tiles) -> DVE-accumulate denominator in f32 ->
     4 y matmuls into PSUM. j-loop software-pipelined (scores of tile j+1
     issue before the exp-dependent y matmuls of tile j). Diagonal blocks
     trimmed at 128-query granularity for i4>0. One ones-matmul per
     (i4,h) turns the f32r accumulator into the softmax denominator.
  D: out^T = W_out_c^T @ y^T with y^T SBUF-resident bf16, W_out resident
     bf16, kc-major for stationary reuse -> outT f32 -> HBM.
Host sums the 8 partial outT (row-split TP gather) and transposes.
"""

import math
from contextlib import ExitStack

import numpy as np
import ml_dtypes

import concourse.bass as bass
import concourse.tile as tile
from concourse import bacc, mybir
from concourse.bass_utils import run_bass_kernel_spmd
from concourse.masks import make_identity

F32 = mybir.dt.float32
F32R = mybir.dt.float32r
BF16 = mybir.dt.bfloat16
AF = mybir.ActivationFunctionType

# Problem constants (hardcoded per harness contract)
T_FULL = 2048
E = 2048          # n_embd
KV = 512          # kv low rank == head size
QL = 1024         # q low rank
RH = 64           # rope head size
QKH = KV + RH     # 576
NH = 16
NCORES = 8
HPC = NH // NCORES
SCALE = 1.0 / math.sqrt(float(KV))

P = 128
LATF = KV + 2 * RH + QL   # 1664 = ckv 512 | kr 64 | pad 64 | cq 1024
NLG = LATF // P           # 13 latent groups
TLOC = T_FULL // NCORES   # 256


def _make_rot64(nc, pool):
    """RT0 [64, 64] f32 permutation with RT0[x, y] = 1 iff x == (y+32) % 64:
    matmul(out, lhsT=RT, rhs=src) gives out[d] = src[(d+32) % 64]."""
    rt0 = pool.tile([RH, RH], F32, tag="rt0")
    nc.gpsimd.memset(rt0[:], 0.0)
    nc.gpsimd.affine_select(
        out=rt0[:], in_=rt0[:], compare_op=mybir.AluOpType.not_equal,
        fill=1.0, base=-32, channel_multiplier=1, pattern=[[-1, RH]],
    )
    nc.gpsimd.affine_select(
        out=rt0[:], in_=rt0[:], compare_op=mybir.AluOpType.not_equal,
        fill=1.0, base=32, channel_multiplier=1, pattern=[[-1, RH]],
    )
    return rt0


def build_kernel(T=T_FULL):
    assert T == T_FULL
    NT512 = T // 512
    NKT = T // P
    EK = E // P            # 16 contraction slices in phase A
    QK = QL // P           # 8 contraction slices in phase B
    DK = HPC * KV // P     # 8 contraction slices in phase D

    nc = bacc.Bacc("TRN2", target_bir_lowering=False, debug=False,
                   num_devices=NCORES)

    xT = nc.dram_tensor("xT", [E, TLOC], BF16, kind="ExternalInput").ap()
    wlat = nc.dram_tensor("wlat", [E, LATF], BF16, kind="ExternalInput").ap()
    wqd = nc.dram_tensor("wqd", [QL, HPC * QKH], BF16, kind="ExternalInput").ap()
    wout = nc.dram_tensor("wout", [HPC * KV, E], BF16, kind="ExternalInput").ap()
    cos2d = nc.dram_tensor("cos2", [P, T], BF16, kind="ExternalInput").ap()
    ssin2d = nc.dram_tensor("ssin2", [P, T], BF16, kind="ExternalInput").ap()
    klcosd = nc.dram_tensor("klcos", [RH, TLOC], BF16, kind="ExternalInput").ap()
    klsind = nc.dram_tensor("klsin", [RH, TLOC], BF16, kind="ExternalInput").ap()
    outT = nc.dram_tensor("outT", [E, T], F32, kind="ExternalOutput").ap()

    with tile.TileContext(nc) as tc, ExitStack() as ctx:
        dram = ctx.enter_context(tc.tile_pool(name="dram", bufs=1, space="DRAM"))
        cst = ctx.enter_context(tc.tile_pool(name="cst", bufs=1))
        pp = ctx.enter_context(tc.tile_pool(name="pp", bufs=2, space="PSUM"))

        stage = dram.tile([LATF, TLOC], BF16)
        agout = dram.tile([NCORES * LATF, TLOC], BF16, addr_space="Shared")
        # [p, group, rank, s] view of the gathered latents
        ag_r = agout[:].rearrange("(r g p) s -> p g r s", p=P, g=NLG)

        # ---- constants ----
        ident0 = cst.tile([P, P], F32, tag="ident0")
        make_identity(nc, ident0[:])
        ident = cst.tile([P, P], BF16, tag="ident")
        nc.vector.tensor_copy(ident[:], ident0[:])
        ones0 = cst.tile([P, 1], F32, tag="ones0")
        nc.gpsimd.memset(ones0[:], 1.0)
        ones_col = cst.tile([P, 1], F32R, tag="ones")
        nc.vector.tensor_copy(ones_col[:], ones0[:])
        rt0 = _make_rot64(nc, cst)
        rot = cst.tile([P, P], BF16, tag="rot")   # block-diag(rot64, rot64)
        nc.gpsimd.memset(rot[:], 0.0)
        nc.gpsimd.dma_start(rot[0:RH, 0:RH], rt0[:])
        nc.gpsimd.dma_start(rot[RH:P, RH:P], rt0[:])
        cos2 = cst.tile([P, T], BF16, tag="cos2")
        nc.sync.dma_start(cos2[:], cos2d[:])
        ssin2 = cst.tile([P, T], BF16, tag="ssin2")
        nc.sync.dma_start(ssin2[:], ssin2d[:])
        klcos = cst.tile([RH, TLOC], BF16, tag="klcos")
        nc.sync.dma_start(klcos[:], klcosd[:])
        klsin = cst.tile([RH, TLOC], BF16, tag="klsin")
        nc.sync.dma_start(klsin[:], klsind[:])
        # stair masks: masks[r][p, q] = 1 iff q >= 128*r + p
        masks = []
        for r in range(4):
            m0 = cst.tile([P, 512], F32, tag=f"mask{r}", name=f"mask{r}")
            nc.gpsimd.memset(m0[:], 1.0)
            nc.gpsimd.affine_select(
                out=m0[:], in_=m0[:], compare_op=mybir.AluOpType.is_ge,
                fill=0.0, base=-P * r, channel_multiplier=-1,
                pattern=[[1, 512]],
            )
            mb = cst.tile([P, 512], BF16, tag=f"maskb{r}", name=f"maskb{r}")
            nc.vector.tensor_copy(mb[:], m0[:])
            masks.append(mb)

        # ============ Phase A-local: latT slice + AllGather =================
        with ExitStack() as actx:
            aw = actx.enter_context(tc.tile_pool(name="aw", bufs=1))
            ast = actx.enter_context(tc.tile_pool(name="ast", bufs=3))

            xloc = aw.tile([P, EK, TLOC], BF16, tag="xloc")
            nc.sync.dma_start(xloc[:], xT.rearrange("(ko p) t -> p ko t", p=P))
            wl_r = wlat.rearrange("(ko p) m -> p ko m", p=P)
            wlt = []
            for g in range(NLG):
                wg = aw.tile([P, EK, P], BF16, tag=f"wlat{g}", name=f"wlat{g}")
                eng = (nc.gpsimd, nc.scalar, nc.sync)[g % 3]
                eng.dma_start(wg[:], wl_r[:, :, g * P : (g + 1) * P])
                wlt.append(wg)

            for g in range(NLG):
                ps = pp.tile([P, TLOC], F32, tag="mm", name="psA")
                for kc in range(EK):
                    nc.tensor.matmul(
                        ps[:], wlt[g][:, kc, :], xloc[:, kc, :],
                        start=(kc == 0), stop=(kc == EK - 1),
                    )
                st = ast.tile([P, TLOC], BF16, tag="ast", name="ast")
                if g == 4:
                    # rows 0:64 = k_r -> rope locally before staging
                    nc.gpsimd.memset(st[RH:P, :], 0.0)
                    kraw = ast.tile([RH, TLOC], BF16, tag="kraw", name="kraw")
                    nc.vector.tensor_copy(kraw[:], ps[0:RH, :])
                    pr = pp.tile([RH, TLOC], F32, tag="mm", name="prk")
                    nc.tensor.matmul(pr[:], rot[0:RH, 0:RH], kraw[:],
                                     start=True, stop=True)
                    nc.vector.tensor_mul(st[0:RH, :], kraw[:], klcos[:])
                    rotk = ast.tile([RH, TLOC], BF16, tag="rotk", name="rotk")
                    nc.vector.tensor_mul(rotk[:], pr[:], klsin[:])
                    nc.vector.tensor_add(st[0:RH, :], st[0:RH, :], rotk[:])
                else:
                    nc.vector.tensor_copy(st[:], ps[:])
                nc.gpsimd.dma_start(stage[g * P : (g + 1) * P, :], st[:])

            nc.gpsimd.collective_compute(
                "AllGather",
                mybir.AluOpType.bypass,
                replica_groups=[list(range(NCORES))],
                ins=[stage.opt()],
                outs=[agout.opt()],
            )

        # ---- persistent SBUF residents (allocated after phase-A frees) ----
        kvp = ctx.enter_context(tc.tile_pool(name="kvp", bufs=1))
        ckvT = kvp.tile([P, KV // P, T], BF16, tag="ckvT")
        # rows 0:64 and 64:128 both hold roped k_r, so head h's score matmul
        # can use lhsT at the same base partition as its packed-qr rhs slice
        krT2 = kvp.tile([P, T], BF16, tag="krT2")
        qnT = [kvp.tile([P, KV // P, T], BF16, tag=f"qnT{h}", name=f"qnT{h}")
               for h in range(HPC)]
        qrT2 = kvp.tile([P, T], BF16, tag="qrT2")   # rows 0:64 h0, 64:128 h1
        v = kvp.tile([P, NKT, KV], BF16, tag="v")
        yT = kvp.tile([P, DK, T], BF16, tag="yT")
        accD = kvp.tile([P, 512], F32R, tag="accD")
        denb = kvp.tile([P, 512], F32, tag="denb")

        # gathered kv readback (one DMA per d-chunk: 3-dim APs each side)
        for dc in range(KV // P):
            nc.sync.dma_start(
                ckvT[:, dc, :].rearrange("p (r s) -> p r s", r=NCORES),
                ag_r[:, dc, :, :],
            )
        nc.sync.dma_start(
            krT2[0:RH, :].rearrange("p (r s) -> p r s", r=NCORES),
            ag_r[0:RH, 4, :, :],
        )
        nc.gpsimd.dma_start(krT2[RH:P, :], krT2[0:RH, :])

        wop = ctx.enter_context(tc.tile_pool(name="wop", bufs=1))
        wout_sb = wop.tile([P, DK, E], BF16, tag="wout")
        nc.scalar.dma_start(
            wout_sb[:], wout.rearrange("(ko p) e -> p ko e", p=P)
        )

        # ============ Phase B: q decode from gathered c_q ===================
        with ExitStack() as bctx:
            bw = bctx.enter_context(tc.tile_pool(name="bw", bufs=1))
            bs = bctx.enter_context(tc.tile_pool(name="bs", bufs=2))

            wqd_sb = bw.tile([P, QK, HPC * QKH], BF16, tag="wqd")
            nc.gpsimd.dma_start(
                wqd_sb[:], wqd.rearrange("(ko p) m -> p ko m", p=P)
            )
            # m-groups: 0 = [qr_h0|qr_h1]; 1-4 = qn_h0; 5-8 = qn_h1
            for tcc in range(NT512):
                qsl = slice(tcc * 512, (tcc + 1) * 512)
                cqt = bs.tile([P, QK, 512], BF16, tag="cqt", name="cqt")
                for r in range(2):
                    nc.sync.dma_start(
                        cqt[:, :, r * TLOC : (r + 1) * TLOC],
                        ag_r[:, 5:13, 2 * tcc + r, :],
                    )
                for gm in range(9):
                    ps = pp.tile([P, 512], F32, tag="mm", name="psB")
                    for kc in range(QK):
                        nc.tensor.matmul(
                            ps[:], wqd_sb[:, kc, gm * P : (gm + 1) * P],
                            cqt[:, kc, :],
                            start=(kc == 0), stop=(kc == QK - 1),
                        )
                    if gm == 0:
                        qraw = bs.tile([P, 512], BF16, tag="qraw", name="qraw")
                        nc.vector.tensor_copy(qraw[:], ps[:])
                        prq = pp.tile([P, 512], F32, tag="mm", name="prq")
                        nc.tensor.matmul(prq[:], rot[:], qraw[:],
                                         start=True, stop=True)
                        nc.vector.tensor_mul(qrT2[:, qsl], qraw[:],
                                             cos2[:, qsl])
                        rotq = bs.tile([P, 512], BF16, tag="rotq", name="rotq")
                        nc.vector.tensor_mul(rotq[:], prq[:], ssin2[:, qsl])
                        nc.vector.tensor_add(qrT2[:, qsl], qrT2[:, qsl],
                                             rotq[:])
                    else:
                        h, dc = (gm - 1) // 4, (gm - 1) % 4
                        if gm % 2 == 1:
                            nc.vector.tensor_copy(qnT[h][:, dc, qsl], ps[:])
                        else:
                            nc.scalar.copy(qnT[h][:, dc, qsl], ps[:])

        # ============ Phase C: attention ====================================
        with ExitStack() as cctx:
            cs = cctx.enter_context(tc.tile_pool(name="cs", bufs=2))

            # v[t, d] via PE transposes of ckvT (ptr closes before ppy opens
            # to stay within the 8 PSUM banks)
            with tc.tile_pool(name="ptr", bufs=2, space="PSUM") as ptr:
                for dc in range(KV // P):
                    for tt in range(NKT):
                        pt = ptr.tile([P, P], BF16, tag="tr", name="pt")
                        nc.tensor.transpose(
                            pt[:], ckvT[:, dc, tt * P : (tt + 1) * P], ident[:]
                        )
                        if tt % 2 == 0:
                            nc.vector.tensor_copy(
                                v[:, tt, dc * P : (dc + 1) * P], pt[:]
                            )
                        else:
                            nc.scalar.copy(
                                v[:, tt, dc * P : (dc + 1) * P], pt[:]
                            )

            ppy = cctx.enter_context(tc.tile_pool(name="ppy", bufs=1,
                                                  space="PSUM"))
            pden = cctx.enter_context(tc.tile_pool(name="pden", bufs=1,
                                                   space="PSUM"))

            for i4 in range(NT512):
                qsl = slice(i4 * 512, (i4 + 1) * 512)
                for h in range(HPC):
                    # tile list: (ksl, off, mask_r); off = query-col offset
                    # into the 512 chunk (free-dim trimming for i4>0)
                    tiles = []
                    if i4 == 0:
                        tiles = [(slice(r * P, (r + 1) * P), 0, r)
                                 for r in range(4)]
                    else:
                        tiles = [(slice(j * P, (j + 1) * P), 0, None)
                                 for j in range(4 * i4)]
                        base = 4 * i4
                        for r in (3, 2, 1, 0):
                            off = r * P if r else 0
                            tiles.append(
                                (slice((base + r) * P, (base + r + 1) * P),
                                 off, r)
                            )
                    nj = len(tiles)
                    psy = [ppy.tile([P, 512], F32, tag=f"y{dc}",
                                    name=f"psy{dc}")
                           for dc in range(KV // P)]
                    q0 = i4 * 512

                    def scores(idx):
                        ksl, off, _ = tiles[idx]
                        qs = slice(q0 + off, q0 + 512)
                        ps = pp.tile([P, 512], F32, tag="mm",
                                     name=f"psS{idx % 2}")
                        for dc in range(KV // P):
                            nc.tensor.matmul(
                                ps[:, off:512], ckvT[:, dc, ksl],
                                qnT[h][:, dc, qs],
                                start=(dc == 0), stop=False,
                            )
                        nc.tensor.matmul(
                            ps[:, off:512],
                            krT2[h * RH : (h + 1) * RH, ksl],
                            qrT2[h * RH : (h + 1) * RH, qs],
                            start=False, stop=True,
                        )
                        return ps

                    def post(idx, ps):
                        ksl, off, mr = tiles[idx]
                        se = cs.tile([P, 512], BF16, tag="se", bufs=3,
                                     name="se")
                        nc.scalar.activation(se[:, off:512], ps[:, off:512],
                                             AF.Exp)
                        if mr is not None:
                            nc.vector.tensor_mul(
                                se[:, off:512], se[:, off:512],
                                masks[mr][:, off:512],
                            )
                        if idx == 0:
                            nc.vector.tensor_copy(accD[:], se[:])
                        else:
                            nc.vector.tensor_add(
                                accD[:, off:512], accD[:, off:512],
                                se[:, off:512],
                            )
                        first = (idx == 0)
                        last = (idx == nj - 1)
                        for dc in range(KV // P):
                            nc.tensor.matmul(
                                psy[dc][:, off:512],
                                v[:, tiles[idx][0].start // P,
                                  dc * P : (dc + 1) * P],
                                se[:, off:512],
                                start=first, stop=last,
                            )

                    prev = scores(0)
                    for idx in range(1, nj):
                        cur = scores(idx)
                        post(idx - 1, prev)
                        prev = cur
                    post(nj - 1, prev)

                    # denominator: colsum(accD) via ones-matmul, then 1/x
                    psden = pden.tile([1, 512], F32, tag="den", name="psden")
                    nc.tensor.matmul(psden[:], ones_col[:], accD[:],
                                     start=True, stop=True)
                    deninv = cs.tile([1, 512], F32, tag="deninv",
                                     name="deninv")
                    nc.vector.reciprocal_approx_fast(out=deninv[:],
                                                     in_=psden[:])
                    nc.gpsimd.partition_broadcast(denb[:], deninv[:])
                    for dc in range(KV // P):
                        nc.vector.tensor_mul(
                            yT[:, h * (KV // P) + dc, qsl], psy[dc][:],
                            denb[:],
                        )

        # ============ Phase D: out^T = W_out_c^T @ y^T ======================
        with ExitStack() as dctx:
            dst = dctx.enter_context(tc.tile_pool(name="dst", bufs=3))
            ppd = dctx.enter_context(tc.tile_pool(name="ppd", bufs=1,
                                                  space="PSUM"))
            for mc in range(E // P):
                psD = [ppd.tile([P, 512], F32, tag=f"d{tcc}",
                                name=f"psD{tcc}")
                       for tcc in range(NT512)]
                for kc in range(DK):
                    for tcc in range(NT512):
                        nc.tensor.matmul(
                            psD[tcc][:],
                            wout_sb[:, kc, mc * P : (mc + 1) * P],
                            yT[:, kc, tcc * 512 : (tcc + 1) * 512],
                            start=(kc == 0), stop=(kc == DK - 1),
                        )
                    if kc == DK - 1:
                        for tcc in range(NT512):
                            ost = dst.tile([P, 512], F32,
                                           tag=f"ost{tcc % 2}",
                                           name="ost")
                            if tcc % 2 == 0:
                                nc.vector.tensor_copy(ost[:], psD[tcc][:])
                            else:
                                nc.scalar.copy(ost[:], psD[tcc][:])
                            (nc.scalar if tcc % 2 else nc.sync).dma_start(
                                outT[mc * P : (mc + 1) * P,
                                     tcc * 512 : (tcc + 1) * 512],
                                ost[:],
                            )

    nc.compile()
    return nc


_NC_CACHE = {}


def _get_nc(T=T_FULL):
    if T not in _NC_CACHE:
        _NC_CACHE[T] = build_kernel(T)
    return _NC_CACHE[T]


def make_in_maps(x, cos, sin, W_qkv, W_qdec, W_out):
    bf = ml_dtypes.bfloat16
    x = np.asarray(x)
    xT = np.ascontiguousarray(x[0].T).astype(bf)           # [E, T]
    W_qkv = np.asarray(W_qkv).astype(np.float32)
    W_qdec = np.asarray(W_qdec).astype(np.float32)
    W_out = np.asarray(W_out).astype(np.float32)
    cos = np.asarray(cos).astype(np.float32)
    sin = np.asarray(sin).astype(np.float32)

    # Wlat columns: ckv 512 | kr 64 | pad 64 | cq 1024  (replicated)
    wlat = np.zeros((E, LATF), np.float32)
    wlat[:, 0:KV] = W_qkv[:, 0:KV]
    wlat[:, KV : KV + RH] = W_qkv[:, KV : KV + RH]
    wlat[:, KV + 2 * RH :] = W_qkv[:, QKH:]
    wlat = wlat.astype(bf)

    cosT = cos.T.copy()                                     # [64, T]
    ssinT = sin.T.copy()
    ssinT[0 : RH // 2] *= -1.0
    cos2 = np.vstack([cosT, cosT]).astype(bf)               # [128, T]
    ssin2 = np.vstack([ssinT, ssinT]).astype(bf)

    Wq = W_qdec * SCALE
    in_maps = []
    for c in range(NCORES):
        h0, h1 = 2 * c, 2 * c + 1
        # wqd cols: [qr_h0 64 | qr_h1 64 | qn_h0 512 | qn_h1 512]
        wqd_c = np.concatenate(
            [
                Wq[:, h0 * QKH + KV : (h0 + 1) * QKH],
                Wq[:, h1 * QKH + KV : (h1 + 1) * QKH],
                Wq[:, h0 * QKH : h0 * QKH + KV],
                Wq[:, h1 * QKH : h1 * QKH + KV],
            ],
            axis=1,
        ).astype(bf)
        tsl = slice(c * TLOC, (c + 1) * TLOC)
        in_maps.append({
            "xT": np.ascontiguousarray(xT[:, tsl]),
            "wlat": wlat,
            "wqd": wqd_c,
            "wout": W_out[c * HPC * KV : (c + 1) * HPC * KV].astype(bf),
            "cos2": cos2,
            "ssin2": ssin2,
            "klcos": np.ascontiguousarray(cos2[0:RH, tsl]),
            "klsin": np.ascontiguousarray(ssin2[0:RH, tsl]),
        })
    return in_maps


def kernel(x, cos, sin, W_qkv, W_qdec, W_out, _trace=False, _tmpdir=None):
    T = np.asarray(x).shape[1]
    nc = _get_nc(T)
    in_maps = make_in_maps(x, cos, sin, W_qkv, W_qdec, W_out)
    res = run_bass_kernel_spmd(
        nc, in_maps, core_ids=list(range(NCORES)),
        trace=_trace, tmpdir=_tmpdir,
    )
    out = np.zeros((E, T), np.float32)
    for r in res.results:
        out += r["outT"]
    kernel.last_results = res
    return np.ascontiguousarray(out.T)[None].astype(np.float32)


# revision 29
# speedup vs baseline: 1.3088x; 1.1304x over previous
"""MLA-style causal self-attention on 8 Trainium2 NeuronCores.

v6: tensor-parallel over heads (2/core) for attention + output projection;
the latent projection (x @ W_qkv) is SHARDED over the sequence across cores
and AllGathered in two pieces (c_q first, so phase B starts as soon as the
first collective lands; ckv/k_r second, hidden under phase B). All matmul
operands are bf16 (full PE rate at any free size, half the DMA/SBUF of
f32r); PSUM accumulation stays fp32.

Per-core device dataflow (everything transposed: contraction on partitions):
  A-local: latT[1664, 256] = Wlat^T @ xT_slice for this core's T-slice.
           Column groups: c_q 1024 (g0-7, staged -> AllGather #1) then
           ckv 512 | k_r 64 + pad (g8-12, k-rope applied locally via a
           64x64 rotate-half permutation matmul, staged -> AllGather #2).
  B: per 512-query chunk, q^T = W_qdec_c^T @ c_q^T from gathered c_q
     (contraction 1024, SCALE folded host-side), rope group first
     ([qr_h0|qr_h1] packed as one 128-row group, block-diag perm matmul).
  C: per (chunk i4, head h): flash-style causal attention, 128-key tiles:
     4 ckv + 1 rope score matmuls -> exp on ACT (bf16) -> stair-mask on
     diagonal tiles -> DVE-accumulated softmax denominator (f32) -> 4 y
     matmuls into PSUM. j-loop software-pipelined two tiles deep. Diagonal
     512-blocks trimmed at 128-query granularity for i4>0 (free-dim
     offsets; last psy matmul is full-width r=0 so accumulation groups
     close cleanly). One ones-matmul per (i4,h) reduces the accumulator
     to the denominator; reciprocal + partition_broadcast -> normalize.
  D: out^T = W_out_c^T @ y^T with y^T and W_out SBUF-resident bf16,
     kc-major for stationary reuse, PSUM rotated over 5 tags to avoid
     write-after-read stalls at mc boundaries -> outT f32 -> HBM.
Host sums the 8 partial outT (row-split TP gather) and transposes.
"""

import math
from contextlib import ExitStack

import numpy as np
import ml_dtypes

import concourse.bass as bass
import concourse.tile as tile
from concourse import bacc, mybir
from concourse.bass_utils import run_bass_kernel_spmd
from concourse.masks import make_identity

F32 = mybir.dt.float32
F32R = mybir.dt.float32r
BF16 = mybir.dt.bfloat16
AF = mybir.ActivationFunctionType

T_FULL = 2048
E = 2048
KV = 512
QL = 1024
RH = 64
QKH = KV + RH     # 576
NH = 16
NCORES = 8
HPC = NH // NCORES
SCALE = 1.0 / math.sqrt(float(KV))

P = 128
LATF = QL + KV + 2 * RH   # 1664 = cq 1024 | ckv 512 | kr 64 | pad 64
NLG = LATF // P           # 13 latent groups
NG1 = QL // P             # 8 groups in AllGather #1 (c_q)
NG2 = NLG - NG1           # 5 groups in AllGather #2 (ckv + kr|pad)
TLOC = T_FULL // NCORES   # 256


def _make_rot64(nc, pool):
    """RT0 [64, 64] f32 permutation: RT0[x, y] = 1 iff x == (y+32) % 64, so
    matmul(out, lhsT=RT, rhs=src) gives out[d] = src[(d+32) % 64]."""
    rt0 = pool.tile([RH, RH], F32, tag="rt0")
    nc.gpsimd.memset(rt0[:], 0.0)
    nc.gpsimd.affine_select(
        out=rt0[:], in_=rt0[:], compare_op=mybir.AluOpType.not_equal,
        fill=1.0, base=-32, channel_multiplier=1, pattern=[[-1, RH]],
    )
    nc.gpsimd.affine_select(
        out=rt0[:], in_=rt0[:], compare_op=mybir.AluOpType.not_equal,
        fill=1.0, base=32, channel_multiplier=1, pattern=[[-1, RH]],
    )
    return rt0


def build_kernel(T=T_FULL):
    assert T == T_FULL
    NT512 = T // 512
    NKT = T // P
    EK = E // P            # 16 contraction slices in phase A
    QK = QL // P           # 8 contraction slices in phase B
    DK = HPC * KV // P     # 8 contraction slices in phase D

    nc = bacc.Bacc("TRN2", target_bir_lowering=False, debug=False,
                   num_devices=NCORES)

    xT = nc.dram_tensor("xT", [E, TLOC], BF16, kind="ExternalInput").ap()
    wlat = nc.dram_tensor("wlat", [E, LATF], BF16, kind="ExternalInput").ap()
    wqd = nc.dram_tensor("wqd", [QL, HPC * QKH], BF16, kind="ExternalInput").ap()
    wout = nc.dram_tensor("wout", [HPC * KV, E], BF16, kind="ExternalInput").ap()
    cos2d = nc.dram_tensor("cos2", [P, T], BF16, kind="ExternalInput").ap()
    ssin2d = nc.dram_tensor("ssin2", [P, T], BF16, kind="ExternalInput").ap()
    klcosd = nc.dram_tensor("klcos", [RH, TLOC], BF16, kind="ExternalInput").ap()
    klsind = nc.dram_tensor("klsin", [RH, TLOC], BF16, kind="ExternalInput").ap()
    outT = nc.dram_tensor("outT", [E, T], F32, kind="ExternalOutput").ap()

    with tile.TileContext(nc) as tc, ExitStack() as ctx:
        dram = ctx.enter_context(tc.tile_pool(name="dram", bufs=1, space="DRAM"))
        cst = ctx.enter_context(tc.tile_pool(name="cst", bufs=1))
        pp = ctx.enter_context(tc.tile_pool(name="pp", bufs=3, space="PSUM"))
        bw = ctx.enter_context(tc.tile_pool(name="bw", bufs=1))
        actx = ExitStack()
        aw = actx.enter_context(tc.tile_pool(name="aw", bufs=1))

        # ---- critical-path input DMAs first, off the sync queue (the sync
        # queue head is the multi-core entry barrier) ----
        xloc = aw.tile([P, EK, TLOC], BF16, tag="xloc")
        nc.scalar.dma_start(xloc[:], xT.rearrange("(ko p) t -> p ko t", p=P))
        wl_r = wlat.rearrange("(ko p) m -> p ko m", p=P)
        wlt = []
        for g in range(NLG):
            wg = aw.tile([P, EK, P], BF16, tag=f"wlat{g}", name=f"wlat{g}")
            eng = (nc.gpsimd, nc.scalar)[g % 2]
            eng.dma_start(wg[:], wl_r[:, :, g * P : (g + 1) * P])
            wlt.append(wg)
        klcos = cst.tile([RH, TLOC], BF16, tag="klcos")
        nc.scalar.dma_start(klcos[:], klcosd[:])
        klsin = cst.tile([RH, TLOC], BF16, tag="klsin")
        nc.scalar.dma_start(klsin[:], klsind[:])

        # phase-B weights on sync: the entry barrier heads that queue, but it
        # clears well before B needs them
        wqd_sb = bw.tile([P, QK, HPC * QKH], BF16, tag="wqd")
        nc.sync.dma_start(
            wqd_sb[:], wqd.rearrange("(ko p) m -> p ko m", p=P)
        )

        # ---- constants needed during phase A (rot for the k-rope at g12) ----
        rt0 = _make_rot64(nc, cst)
        rot = cst.tile([P, P], BF16, tag="rot")   # block-diag(rot64, rot64)
        nc.gpsimd.memset(rot[:], 0.0)
        nc.gpsimd.dma_start(rot[0:RH, 0:RH], rt0[:])
        nc.gpsimd.dma_start(rot[RH:P, RH:P], rt0[:])
        cos2 = cst.tile([P, T], BF16, tag="cos2")
        nc.sync.dma_start(cos2[:], cos2d[:])
        ssin2 = cst.tile([P, T], BF16, tag="ssin2")
        nc.sync.dma_start(ssin2[:], ssin2d[:])

        stage1 = dram.tile([QL, TLOC], BF16)
        ag1out = dram.tile([NCORES * QL, TLOC], BF16, addr_space="Shared")
        stage2 = dram.tile([NG2 * P, TLOC], BF16)
        ag2out = dram.tile([NCORES * NG2 * P, TLOC], BF16, addr_space="Shared")
        ag1_r = ag1out[:].rearrange("(r g p) s -> p g r s", p=P, g=NG1)
        ag2_r = ag2out[:].rearrange("(r g p) s -> p g r s", p=P, g=NG2)

        # ============ Phase A-local: latT slice + AllGathers ================
        if True:
            ast = actx.enter_context(tc.tile_pool(name="ast", bufs=3))
            for g in range(NLG):
                ps = pp.tile([P, TLOC], F32, tag="mm", name="psA")
                for kc in range(EK):
                    nc.tensor.matmul(
                        ps[:], wlt[g][:, kc, :], xloc[:, kc, :],
                        start=(kc == 0), stop=(kc == EK - 1),
                    )
                st = ast.tile([P, TLOC], BF16, tag="ast", name="ast")
                if g == NLG - 1:
                    # rows 0:64 = k_r -> rope locally before staging
                    nc.gpsimd.memset(st[RH:P, :], 0.0)
                    kraw = ast.tile([RH, TLOC], BF16, tag="kraw", name="kraw")
                    nc.vector.tensor_copy(kraw[:], ps[0:RH, :])
                    pr = pp.tile([RH, TLOC], F32, tag="mm", name="prk")
                    nc.tensor.matmul(pr[:], rot[0:RH, 0:RH], kraw[:],
                                     start=True, stop=True)
                    nc.vector.tensor_mul(st[0:RH, :], kraw[:], klcos[:])
                    rotk = ast.tile([RH, TLOC], BF16, tag="rotk", name="rotk")
                    nc.vector.tensor_mul(rotk[:], pr[:], klsin[:])
                    nc.vector.tensor_add(st[0:RH, :], st[0:RH, :], rotk[:])
                else:
                    nc.vector.tensor_copy(st[:], ps[:])
                if g < NG1:
                    nc.gpsimd.dma_start(stage1[g * P : (g + 1) * P, :], st[:])
                    if g == NG1 - 1:
                        # gpsimd blocks on this until AG1 completes, so
                        # stage2 writes go on scalar
                        nc.gpsimd.collective_compute(
                            "AllGather", mybir.AluOpType.bypass,
                            replica_groups=[list(range(NCORES))],
                            ins=[stage1.opt()], outs=[ag1out.opt()],
                        )
                else:
                    nc.scalar.dma_start(
                        stage2[(g - NG1) * P : (g - NG1 + 1) * P, :], st[:]
                    )
            nc.gpsimd.collective_compute(
                "AllGather", mybir.AluOpType.bypass,
                replica_groups=[list(range(NCORES))],
                ins=[stage2.opt()], outs=[ag2out.opt()],
            )
        actx.close()

        # ---- persistent SBUF residents ----
        kvp = ctx.enter_context(tc.tile_pool(name="kvp", bufs=1))
        ckvT = kvp.tile([P, KV // P, T], BF16, tag="ckvT")
        # rows 0:64 and 64:128 both hold roped k_r, so head h's score matmul
        # uses lhsT at the same base partition as its packed-qr rhs slice
        krT2 = kvp.tile([P, T], BF16, tag="krT2")
        qnT = [kvp.tile([P, KV // P, T], BF16, tag=f"qnT{h}", name=f"qnT{h}")
               for h in range(HPC)]
        qrT2 = kvp.tile([P, T], BF16, tag="qrT2")   # rows 0:64 h0, 64:128 h1
        v = kvp.tile([P, NKT, KV], BF16, tag="v")
        yT = kvp.tile([P, DK, T], BF16, tag="yT")
        accD = kvp.tile([P, 512], F32R, tag="accD")
        denb = kvp.tile([P, 512], F32, tag="denb")

        # late constants (needed from phase C on); on gpsimd these execute
        # after the AG2 trigger unblocks, well before C
        ident0 = cst.tile([P, P], F32, tag="ident0")
        make_identity(nc, ident0[:])
        ident = cst.tile([P, P], BF16, tag="ident")
        nc.vector.tensor_copy(ident[:], ident0[:])
        ones0 = cst.tile([P, 1], F32, tag="ones0")
        nc.gpsimd.memset(ones0[:], 1.0)
        ones_col = cst.tile([P, 1], F32R, tag="ones")
        nc.vector.tensor_copy(ones_col[:], ones0[:])
        masks = []
        for r in range(4):
            m0 = cst.tile([P, 512], F32, tag=f"mask{r}", name=f"mask{r}")
            nc.gpsimd.memset(m0[:], 1.0)
            nc.gpsimd.affine_select(
                out=m0[:], in_=m0[:], compare_op=mybir.AluOpType.is_ge,
                fill=0.0, base=-P * r, channel_multiplier=-1,
                pattern=[[1, 512]],
            )
            mb = cst.tile([P, 512], BF16, tag=f"maskb{r}", name=f"maskb{r}")
            nc.vector.tensor_copy(mb[:], m0[:])
            masks.append(mb)

        wop = ctx.enter_context(tc.tile_pool(name="wop", bufs=1))
        wout_sb = wop.tile([P, DK, E], BF16, tag="wout")
        nc.gpsimd.dma_start(
            wout_sb[:], wout.rearrange("(ko p) e -> p ko e", p=P)
        )

        # ============ Phase B: q decode from gathered c_q ===================
        with ExitStack() as bctx:
            bs = bctx.enter_context(tc.tile_pool(name="bs", bufs=2))
            # m-groups: 0 = [qr_h0|qr_h1]; 1-4 = qn_h0; 5-8 = qn_h1
            for tcc in range(NT512):
                qsl = slice(tcc * 512, (tcc + 1) * 512)
                cqt = bs.tile([P, QK, 512], BF16, tag="cqt", name="cqt")
                for r in range(2):
                    nc.sync.dma_start(
                        cqt[:, :, r * TLOC : (r + 1) * TLOC],
                        ag1_r[:, :, 2 * tcc + r, :],
                    )
                if tcc == NT512 - 1:
                    # kv readbacks go behind the last cqt load on sync so
                    # they never delay a cqt prefetch (they block on AG2)
                    for dc in range(KV // P):
                        nc.sync.dma_start(
                            ckvT[:, dc, :].rearrange("p (r s) -> p r s",
                                                     r=NCORES),
                            ag2_r[:, dc, :, :],
                        )
                    nc.sync.dma_start(
                        krT2[0:RH, :].rearrange("p (r s) -> p r s",
                                                r=NCORES),
                        ag2_r[0:RH, 4, :, :],
                    )
                    nc.gpsimd.dma_start(krT2[RH:P, :], krT2[0:RH, :])
                for gm in range(9):
                    ps = pp.tile([P, 512], F32, tag="mm", name="psB")
                    for kc in range(QK):
                        nc.tensor.matmul(
                            ps[:], wqd_sb[:, kc, gm * P : (gm + 1) * P],
                            cqt[:, kc, :],
                            start=(kc == 0), stop=(kc == QK - 1),
                        )
                    if gm == 0:
                        qraw = bs.tile([P, 512], BF16, tag="qraw", name="qraw")
                        nc.vector.tensor_copy(qraw[:], ps[:])
                        prq = pp.tile([P, 512], F32, tag="mm", name="prq")
                        nc.tensor.matmul(prq[:], rot[:], qraw[:],
                                         start=True, stop=True)
                        nc.vector.tensor_mul(qrT2[:, qsl], qraw[:],
                                             cos2[:, qsl])
                        rotq = bs.tile([P, 512], BF16, tag="rotq", name="rotq")
                        nc.vector.tensor_mul(rotq[:], prq[:], ssin2[:, qsl])
                        nc.vector.tensor_add(qrT2[:, qsl], qrT2[:, qsl],
                                             rotq[:])
                    else:
                        h, dc = (gm - 1) // 4, (gm - 1) % 4
                        if gm % 2 == 1:
                            nc.vector.tensor_copy(qnT[h][:, dc, qsl], ps[:])
                        else:
                            nc.scalar.copy(qnT[h][:, dc, qsl], ps[:])

        # ============ Phase C: attention ====================================
        with ExitStack() as cctx:
            cs = cctx.enter_context(tc.tile_pool(name="cs", bufs=2))

            # v[t, d] via PE transposes of ckvT (scoped so the 2 PSUM banks
            # free before ppy opens)
            with tc.tile_pool(name="ptr", bufs=2, space="PSUM") as ptr:
                for dc in range(KV // P):
                    for tt in range(NKT):
                        pt = ptr.tile([P, P], BF16, tag="tr", name="pt")
                        nc.tensor.transpose(
                            pt[:], ckvT[:, dc, tt * P : (tt + 1) * P], ident[:]
                        )
                        if tt % 2 == 0:
                            nc.vector.tensor_copy(
                                v[:, tt, dc * P : (dc + 1) * P], pt[:]
                            )
                        else:
                            nc.scalar.copy(
                                v[:, tt, dc * P : (dc + 1) * P], pt[:]
                            )

            ppy = cctx.enter_context(tc.tile_pool(name="ppy", bufs=1,
                                                  space="PSUM"))
            pden = cctx.enter_context(tc.tile_pool(name="pden", bufs=1,
                                                   space="PSUM"))

            for i4 in range(NT512):
                for h in range(HPC):
                    # (ksl, off, mask_r): off = query-column offset into the
                    # 512-chunk (free-dim trimming of diagonal blocks)
                    if i4 == 0:
                        tiles = [(slice(r * P, (r + 1) * P), 0, r)
                                 for r in range(4)]
                    else:
                        tiles = [(slice(j * P, (j + 1) * P), 0, None)
                                 for j in range(4 * i4)]
                        base = 4 * i4
                        for r in (3, 2, 1, 0):
                            tiles.append(
                                (slice((base + r) * P, (base + r + 1) * P),
                                 r * P if r else 0, r)
                            )
                    nj = len(tiles)
                    psy = [ppy.tile([P, 512], F32, tag=f"y{dc}",
                                    name=f"psy{dc}")
                           for dc in range(KV // P)]
                    q0 = i4 * 512

                    def scores(idx):
                        ksl, off, _ = tiles[idx]
                        qs = slice(q0 + off, q0 + 512)
                        ps = pp.tile([P, 512], F32, tag="mm",
                                     name=f"psS{idx % 3}")
                        for dc in range(KV // P):
                            nc.tensor.matmul(
                                ps[:, off:512], ckvT[:, dc, ksl],
                                qnT[h][:, dc, qs],
                                start=(dc == 0), stop=False,
                            )
                        nc.tensor.matmul(
                            ps[:, off:512],
                            krT2[h * RH : (h + 1) * RH, ksl],
                            qrT2[h * RH : (h + 1) * RH, qs],
                            start=False, stop=True,
                        )
                        return ps

                    def post(idx, ps):
                        ksl, off, mr = tiles[idx]
                        se = cs.tile([P, 512], BF16, tag="se", bufs=4,
                                     name="se")
                        nc.scalar.activation(se[:, off:512], ps[:, off:512],
                                             AF.Exp)
                        if mr is not None:
                            nc.vector.tensor_mul(
                                se[:, off:512], se[:, off:512],
                                masks[mr][:, off:512],
                            )
                        if idx == 0:
                            nc.vector.tensor_copy(accD[:], se[:])
                        else:
                            nc.vector.tensor_add(
                                accD[:, off:512], accD[:, off:512],
                                se[:, off:512],
                            )
                        first = (idx == 0)
                        last = (idx == nj - 1)
                        for dc in range(KV // P):
                            nc.tensor.matmul(
                                psy[dc][:, off:512],
                                v[:, ksl.start // P, dc * P : (dc + 1) * P],
                                se[:, off:512],
                                start=first, stop=last,
                            )

                    # two-deep software pipeline: scores run two tiles ahead
                    # of the exp-dependent work
                    ring = [scores(0)]
                    if nj > 1:
                        ring.append(scores(1))
                    for idx in range(2, nj):
                        ring.append(scores(idx))
                        post(idx - 2, ring.pop(0))
                    while ring:
                        post(nj - len(ring), ring.pop(0))

                    psden = pden.tile([1, 512], F32, tag="den", name="psden")
                    nc.tensor.matmul(psden[:], ones_col[:], accD[:],
                                     start=True, stop=True)
                    deninv = cs.tile([1, 512], F32, tag="deninv",
                                     name="deninv")
                    nc.vector.reciprocal_approx_fast(out=deninv[:],
                                                     in_=psden[:])
                    nc.gpsimd.partition_broadcast(denb[:], deninv[:])
                    qsl = slice(i4 * 512, (i4 + 1) * 512)
                    for dc in range(KV // P):
                        nc.vector.tensor_mul(
                            yT[:, h * (KV // P) + dc, qsl], psy[dc][:],
                            denb[:],
                        )

        # ============ Phase D: out^T = W_out_c^T @ y^T ======================
        with ExitStack() as dctx:
            dst = dctx.enter_context(tc.tile_pool(name="dst", bufs=3))
            ppd = dctx.enter_context(tc.tile_pool(name="ppd", bufs=1,
                                                  space="PSUM"))
            for mc in range(E // P):
                psD = [ppd.tile([P, 512], F32,
                                tag=f"d{(mc * NT512 + tcc) % 5}",
                                name=f"psD{tcc}")
                       for tcc in range(NT512)]
                for kc in range(DK):
                    for tcc in range(NT512):
                        nc.tensor.matmul(
                            psD[tcc][:],
                            wout_sb[:, kc, mc * P : (mc + 1) * P],
                            yT[:, kc, tcc * 512 : (tcc + 1) * 512],
                            start=(kc == 0), stop=(kc == DK - 1),
                        )
                    if kc == DK - 1:
                        for tcc in range(NT512):
                            ost = dst.tile([P, 512], F32,
                                           tag=f"ost{tcc % 2}",
                                           name="ost")
                            if tcc % 2 == 0:
                                nc.vector.tensor_copy(ost[:], psD[tcc][:])
                            else:
                                nc.scalar.copy(ost[:], psD[tcc][:])
                            (nc.scalar if tcc % 2 else nc.sync).dma_start(
                                outT[mc * P : (mc + 1) * P,
                                     tcc * 512 : (tcc + 1) * 512],
                                ost[:],
                            )

    nc.compile()
    return nc


_NC_CACHE = {}


def _get_nc(T=T_FULL):
    if T not in _NC_CACHE:
        _NC_CACHE[T] = build_kernel(T)
    return _NC_CACHE[T]


def make_in_maps(x, cos, sin, W_qkv, W_qdec, W_out):
    bf = ml_dtypes.bfloat16
    x = np.asarray(x)
    xT = np.ascontiguousarray(x[0].T).astype(bf)           # [E, T]
    W_qkv = np.asarray(W_qkv).astype(np.float32)
    W_qdec = np.asarray(W_qdec).astype(np.float32)
    W_out = np.asarray(W_out).astype(np.float32)
    cos = np.asarray(cos).astype(np.float32)
    sin = np.asarray(sin).astype(np.float32)

    # Wlat columns: cq 1024 | ckv 512 | kr 64 | pad 64  (replicated)
    wlat = np.zeros((E, LATF), np.float32)
    wlat[:, 0:QL] = W_qkv[:, QKH:]
    wlat[:, QL : QL + KV] = W_qkv[:, 0:KV]
    wlat[:, QL + KV : QL + KV + RH] = W_qkv[:, KV : KV + RH]
    wlat = wlat.astype(bf)

    cosT = cos.T.copy()                                     # [64, T]
    ssinT = sin.T.copy()
    ssinT[0 : RH // 2] *= -1.0
    cos2 = np.vstack([cosT, cosT]).astype(bf)               # [128, T]
    ssin2 = np.vstack([ssinT, ssinT]).astype(bf)

    Wq = W_qdec * SCALE
    in_maps = []
    for c in range(NCORES):
        h0, h1 = 2 * c, 2 * c + 1
        # wqd cols: [qr_h0 64 | qr_h1 64 | qn_h0 512 | qn_h1 512]
        wqd_c = np.concatenate(
            [
                Wq[:, h0 * QKH + KV : (h0 + 1) * QKH],
                Wq[:, h1 * QKH + KV : (h1 + 1) * QKH],
                Wq[:, h0 * QKH : h0 * QKH + KV],
                Wq[:, h1 * QKH : h1 * QKH + KV],
            ],
            axis=1,
        ).astype(bf)
        tsl = slice(c * TLOC, (c + 1) * TLOC)
        in_maps.append({
            "xT": np.ascontiguousarray(xT[:, tsl]),
            "wlat": wlat,
            "wqd": wqd_c,
            "wout": W_out[c * HPC * KV : (c + 1) * HPC * KV].astype(bf),
            "cos2": cos2,
            "ssin2": ssin2,
            "klcos": np.ascontiguousarray(cos2[0:RH, tsl]),
            "klsin": np.ascontiguousarray(ssin2[0:RH, tsl]),
        })
    return in_maps


def kernel(x, cos, sin, W_qkv, W_qdec, W_out, _trace=False, _tmpdir=None):
    T = np.asarray(x).shape[1]
    nc = _get_nc(T)
    in_maps = make_in_maps(x, cos, sin, W_qkv, W_qdec, W_out)
    res = run_bass_kernel_spmd(
        nc, in_maps, core_ids=list(range(NCORES)),
        trace=_trace, tmpdir=_tmpdir,
    )
    out = np.zeros((E, T), np.float32)
    for r in res.results:
        out += r["outT"]
    kernel.last_results = res
    return np.ascontiguousarray(out.T)[None].astype(np.float32)
